# revision 28
# baseline (speedup 1.0000x reference)
"""Trainium2 Bass kernel for multi-head attention (B=4, N=2048, DIM=768, H=12).

Sharding: 8 cores; core c handles batch b = c//2 and heads h0 = 6*(c%2) .. h0+5
(tensor-parallel over heads within a batch pair). Each core computes a partial
projection output; the host sums the two partials per batch (proj_b is folded
into the even core's partial on device).

Key compaction: attention_mask zeroes ~half the keys, and masked keys
contribute exactly 0 after softmax. The host gathers only the unmasked keys'
x rows (padded to KT*128), so K/V/S/exp/PV all shrink ~2x. Padded key slots
get an exp bias of -BIG via mbias (per-partition ACT bias), so they vanish
from both the numerator and the ones-column denominator.

Device-side layout (per core):
  - Q^T: [384, 2048] f32r as 3 SBUF tiles of [128, 2048] (2 heads per tile);
    K^T: [384, NK] likewise. V: [NK, 390] f32r as KT tiles of [128, 390];
    per head 65 cols = [V | 1], so the attention matmul also produces the
    softmax denominator in row 64.
  - S^T = K @ Q^T per (k-tile, head, q-block) in PSUM (f32r, 64-contraction);
    exp with per-partition padding bias + 1/8 scale fused in one ACT
    instruction over [128, 1024] (2 heads x 512-q-block, same k-tile).
  - P^T @ V f32r with accumulating matmuls; accumulator copied to SBUF to
    free its PSUM bank, then divided by the denominator (reciprocal +
    GPSIMD partition-broadcast + multiply).
  - proj: out[n, e] = attnT.T @ proj_w_slice.T in f32r; proj_b and v_bias
    added during DVE evictions; partials summed host-side across core pairs.
  - PSUM (8 banks): S tiles 2x2 banks (double-buffered, S emitted one
    iteration ahead to keep ACT fed), 2 attention accumulators, 2 banks for
    the QKV/V/proj rotation. All QKV/V/proj matmul groups stream just-in-time
    into the attention iterations as low-priority fillers.
proj matmuls run as float32r (TF32); host pre-rounds f32r inputs to TF32.

(Tried and rejected: fp8e4 DoubleRow for S — the ISA-mandated [32, 2, m]
weight repack needs cross-partition DMAs and DR disables fast weight load,
so real hardware ran 44us SLOWER than f32r despite 2x fewer matmul cycles.
fp8 P/V for the attention-value matmul fails the 2e-2 error budget.)
"""

import math
import numpy as np
from contextlib import ExitStack

import concourse.bass as bass
import concourse.tile as tile
from concourse import bacc, mybir
from concourse.alu_op_type import AluOpType
from concourse.bass_utils import run_bass_kernel_spmd

N_CORES = 8
B, N, DIM = 4, 2048, 768
NHEADS, HD = 12, 64
HPC = 6              # heads per core
DPC = HPC * HD       # 384 channels per core
CC = DIM // 128      # 6 contraction chunks of 128
QB = 512             # q block width
NQB = N // QB        # 4 q blocks
KB = 384             # k eviction block width
VW = HPC * (HD + 1)  # V width incl per-head ones column (390)
BIG = 80.0
F32 = mybir.dt.float32
F32R = mybir.dt.float32r
BF16 = mybir.dt.bfloat16
FP8 = mybir.dt.float8e4
AF = mybir.ActivationFunctionType
DR = mybir.MatmulPerfMode.DoubleRow

_CACHE = {}


def _tf32_round(a: np.ndarray) -> np.ndarray:
    """Round fp32 to TF32 (10-bit mantissa, round-to-nearest-even)."""
    u = np.ascontiguousarray(a, np.float32).view(np.uint32)
    r = (u + np.uint32(0x0FFF) + ((u >> np.uint32(13)) & np.uint32(1))) \
        & np.uint32(0xFFFFE000)
    return r.view(np.float32)


def _bf16(a: np.ndarray) -> np.ndarray:
    import ml_dtypes
    return np.ascontiguousarray(np.asarray(a, np.float32)).astype(
        ml_dtypes.bfloat16)


def build(reps=1, KT=9):
    NK = KT * 128        # padded kept-key count
    NKB = NK // KB       # k eviction blocks
    nc = bacc.Bacc("TRN2", target_bir_lowering=False, debug=False,
                   num_devices=N_CORES)

    xT = nc.dram_tensor("xT", [DIM, N], F32R, kind="ExternalInput").ap()
    xkT = nc.dram_tensor("xkT", [DIM, NK], F32R, kind="ExternalInput").ap()
    wqT = nc.dram_tensor("wqT", [DIM, DPC], F32R, kind="ExternalInput").ap()
    wkT = nc.dram_tensor("wkT", [DIM, DPC], F32R, kind="ExternalInput").ap()
    wvT = nc.dram_tensor("wvT", [DIM, VW], F32R, kind="ExternalInput").ap()
    qbias = nc.dram_tensor("qbias", [128, 3], F32, kind="ExternalInput").ap()
    mbias = nc.dram_tensor("mbias", [128, KT], F32, kind="ExternalInput").ap()
    pwT = nc.dram_tensor("pwT", [DPC, DIM], F32R, kind="ExternalInput").ap()
    pbb = nc.dram_tensor("pbb", [128, DIM], F32, kind="ExternalInput").ap()
    vbb = nc.dram_tensor("vbb", [128, VW], F32, kind="ExternalInput").ap()
    out = nc.dram_tensor("out", [N, DIM], F32, kind="ExternalOutput").ap()

    with tile.TileContext(nc) as tc, ExitStack() as ctx:
        # ---- pools ----
        # One shared PSUM pool, exactly 8 banks:
        #   tag "s":  [128, 1024] x2 bufs = 4 banks (S^T tiles)
        #   tag "o":  [65, 512]   x2 bufs = 2 banks (attention accumulators)
        #   tag "pj": [128, <=512] x2     = 2 banks (QK/V/proj psum rotation)
        psum = ctx.enter_context(tc.tile_pool(name="psum", bufs=2, space="PSUM"))
        persist = ctx.enter_context(tc.tile_pool(name="persist", bufs=1))
        qt_sb = [persist.tile([128, N], F32R, tag=f"qt{j}", name=f"qt{j}")
                 for j in range(3)]
        kt_sb = [persist.tile([128, NK], F32R, tag=f"kt{j}", name=f"kt{j}")
                 for j in range(3)]
        v_sb = [persist.tile([128, VW], F32R, tag=f"v{t}", name=f"v{t}")
                for t in range(KT)]
        consts = ctx.enter_context(tc.tile_pool(name="consts", bufs=1))
        qb_sb = consts.tile([128, 3], F32, tag="qb", name="qb")
        mb_sb = consts.tile([128, KT], F32, tag="mb", name="mb")
        pbb_sb = consts.tile([128, DIM], F32, tag="pbb", name="pbb")
        vbb_sb = consts.tile([128, VW], F32, tag="vbb", name="vbb")
        pw_sb = [consts.tile([128, DIM], F32R, tag=f"pw{j}", name=f"pw{j}")
                 for j in range(3)]
        xw = ctx.enter_context(tc.tile_pool(name="xw", bufs=1))
        x_sb = [xw.tile([128, N], F32R, tag=f"x{c}", name=f"x{c}")
                for c in range(CC)]
        xk_sb = [xw.tile([128, NK], F32R, tag=f"xk{c}", name=f"xk{c}")
                 for c in range(CC)]
        wq_sb = [xw.tile([128, DPC], F32R, tag=f"wq{c}", name=f"wq{c}")
                 for c in range(CC)]
        wk_sb = [xw.tile([128, DPC], F32R, tag=f"wk{c}", name=f"wk{c}")
                 for c in range(CC)]
        wv_sb = [xw.tile([128, VW], F32R, tag=f"wv{c}", name=f"wv{c}")
                 for c in range(CC)]
        ppool = ctx.enter_context(tc.tile_pool(name="ppool", bufs=4))
        atpool = ctx.enter_context(tc.tile_pool(name="atpool", bufs=1))
        dpool = ctx.enter_context(tc.tile_pool(name="dpool", bufs=2))
        opool = ctx.enter_context(tc.tile_pool(name="opool", bufs=2))

        # ---- input DMAs ----
        nc.sync.dma_start(qb_sb[:], qbias)
        nc.sync.dma_start(mb_sb[:], mbias)
        nc.sync.dma_start(pbb_sb[:], pbb)
        nc.sync.dma_start(vbb_sb[:], vbb)
        for j in range(3):
            nc.sync.dma_start(pw_sb[j][:], pwT[j * 128:(j + 1) * 128, :])
        for c in range(CC):
            nc.sync.dma_start(xk_sb[c][:], xkT[c * 128:(c + 1) * 128, :])
            nc.sync.dma_start(x_sb[c][:], xT[c * 128:(c + 1) * 128, :])
        for c in range(CC):
            nc.sync.dma_start(wk_sb[c][:], wkT[c * 128:(c + 1) * 128, :])
            nc.sync.dma_start(wq_sb[c][:], wqT[c * 128:(c + 1) * 128, :])
        for c in range(CC):
            nc.sync.dma_start(wv_sb[c][:], wvT[c * 128:(c + 1) * 128, :])

        _qk_open = {}

        def emit_q_half(j, nb, half):
            # half of a [128, 512] Q chunk (3 contraction steps) through
            # the 1-bank pj rotation; finer PE granules between attention iters
            dsl = bass.ts(j, 128)
            nsl = bass.ts(nb, QB)
            if half == 0:
                ps = psum.tile([128, QB], F32, tag="pj", name=f"q{j}_{nb}")
                _qk_open[("q", j, nb)] = ps
            else:
                ps = _qk_open.pop(("q", j, nb))
            for c in range(3 * half, 3 * half + 3):
                nc.tensor.matmul(ps[:], wq_sb[c][:, dsl], x_sb[c][:, nsl],
                                 start=(c == 0), stop=(c == CC - 1))
            if half == 1:
                nc.vector.tensor_scalar(qt_sb[j][:, nsl], ps[:],
                                        qb_sb[:, j:j + 1], None,
                                        op0=AluOpType.add)

        def emit_q_group(j, nb):
            emit_q_half(j, nb, 0)
            emit_q_half(j, nb, 1)

        def emit_k_half(j, nb, half):
            dsl = bass.ts(j, 128)
            nsl = bass.ts(nb, KB)
            if half == 0:
                ps = psum.tile([128, KB], F32, tag="pj", name=f"k{j}_{nb}")
                _qk_open[("k", j, nb)] = ps
            else:
                ps = _qk_open.pop(("k", j, nb))
            for c in range(3 * half, 3 * half + 3):
                nc.tensor.matmul(ps[:], wk_sb[c][:, dsl], xk_sb[c][:, nsl],
                                 start=(c == 0), stop=(c == CC - 1))
            if half == 1:
                nc.vector.tensor_copy(kt_sb[j][:, nsl], ps[:])

        def emit_k_group(j, nb):
            emit_k_half(j, nb, 0)
            emit_k_half(j, nb, 1)

        def emit_v(t):
            tsl = bass.ts(t, 128)
            psv = psum.tile([128, VW], F32, tag="pj", name=f"v{t}")
            for c in range(CC):
                nc.tensor.matmul(psv[:], xk_sb[c][:, tsl], wv_sb[c][:],
                                 start=(c == 0), stop=(c == CC - 1))
            nc.vector.tensor_tensor(v_sb[t][:], psv[:], vbb_sb[:],
                                    op=AluOpType.add)

        _pending_div = []

        def emit_attn(qb, j, fillers=()):
            # heads 2j, 2j+1 over q block qb: S^T -> exp -> P^T V -> divide
            fillers = list(fillers)
            qsl = bass.ds(qb * QB, QB)
            at_t = atpool.tile([128, QB], F32R, tag=f"at{j}", name=f"at{qb}_{j}")
            o_ps = [psum.tile([HD + 1, QB], F32, tag="o", name=f"o{qb}_{j}_{i}")
                    for i in range(2)]
            s_tiles = {}

            def emit_s(kt):
                ksl = bass.ts(kt, 128)
                s = psum.tile([128, 2 * QB], F32, tag="s",
                              name=f"s{qb}_{j}_{kt}")
                for hh in range(2):
                    hsl = bass.ds(hh * 64, 64)
                    nc.tensor.matmul(s[:, bass.ts(hh, QB)],
                                     kt_sb[j][hsl, ksl], qt_sb[j][hsl, qsl],
                                     start=True, stop=True)
                s_tiles[kt] = s

            emit_s(0)
            for d in _pending_div:
                d()
            _pending_div.clear()
            for kt in range(KT):
                if kt + 1 < KT:
                    emit_s(kt + 1)   # one iteration ahead: keeps ACT fed
                s = s_tiles.pop(kt)
                p = ppool.tile([128, 2 * QB], F32R, tag="p", name=f"p{qb}_{j}_{kt}")
                nc.scalar.activation(p[:], s[:], AF.Exp,
                                     bias=mb_sb[:, kt:kt + 1], scale=0.125)
                for hh in range(2):
                    vsl = bass.ds((2 * j + hh) * (HD + 1), HD + 1)
                    nc.tensor.matmul(o_ps[hh], v_sb[kt][:, vsl],
                                     p[:, bass.ts(hh, QB)],
                                     start=(kt == 0), stop=(kt == KT - 1))
                if fillers:
                    fillers.pop(0)()
            for f in fillers:
                f()
            def _division():
                for hh in range(2):
                    ocp = dpool.tile([HD + 1, QB], F32, tag="ocp", name="ocp")
                    nc.vector.tensor_copy(ocp[:], o_ps[hh])  # frees the bank
                    rd = dpool.tile([1, QB], F32, tag="rd", name="rd")
                    bc = dpool.tile([64, QB], F32, tag="bc", name="bc")
                    nc.vector.reciprocal(rd[:], ocp[HD:HD + 1, :])
                    nc.gpsimd.partition_broadcast(bc[:], rd[:])
                    nc.vector.tensor_tensor(at_t[bass.ds(hh * 64, 64), :],
                                            ocp[0:HD, :], bc[:],
                                            op=AluOpType.mult)
            _pending_div.append(_division)
            return at_t

        def emit_proj_tile(t, qb, at_tiles):
            tsl = bass.ts(t, 128)
            po = [psum.tile([128, 384], F32, tag="pj", name=f"po{t}_{e}")
                  for e in range(2)]
            for e in range(2):
                esl = bass.ts(e, 384)
                for j in range(3):
                    nc.tensor.matmul(po[e],
                                     at_tiles[j][:, bass.ts(t - 4 * qb, 128)],
                                     pw_sb[j][:, esl],
                                     start=(j == 0), stop=(j == 2))
            for e in range(2):
                ot = opool.tile([128, 384], F32, tag="ot", name=f"ot{t}_{e}")
                nc.vector.tensor_tensor(ot[:], po[e],
                                        pbb_sb[:, bass.ts(e, 384)],
                                        op=AluOpType.add)
                nc.sync.dma_start(out[tsl, bass.ts(e, 384)], ot[:])

        # ---- emission: sprinkle QKV/proj groups into the attention stream
        import functools
        carry = None
        for _rep in range(reps):
            if KT == 9:
                # minimal serial prologue: K0 block 0 only, Q0 block 0,
                # V0-V2; K0 blocks 1/2 stream as fillers ahead of their
                # first consumers (S(kt=3) emitted at iteration 2, S(kt=6)
                # at iteration 5; V(t) consumed at iteration t).
                emit_k_group(0, 0)
                emit_q_group(0, 0)
                for t in range(3):
                    emit_v(t)
                fill0 = [functools.partial(emit_k_group, 0, 1),
                         functools.partial(emit_v, 3),
                         functools.partial(emit_v, 4),
                         functools.partial(emit_k_group, 0, 2),
                         functools.partial(emit_v, 5),
                         functools.partial(emit_v, 6),
                         functools.partial(emit_v, 7),
                         functools.partial(emit_v, 8),
                         functools.partial(emit_k_group, 1, 0)]
                k1_start = 1
            else:
                # conservative prologue for other KT
                for nb in range(NKB):
                    emit_k_group(0, nb)
                emit_q_group(0, 0)
                emit_v(0)
                emit_v(1)
                fill0 = [functools.partial(emit_v, t) for t in range(2, KT)]
                fill0 += [functools.partial(emit_k_group, 1, nb)
                          for nb in range(2)]
                k1_start = 2
            if carry is not None:
                # software-pipelined rep boundary: the previous rep's final
                # proj tiles run as post-loop fillers here (PE-slack window
                # while ACT works through this attn's exps); its last
                # division crosses via _pending_div into this attn's start.
                fill0 += [functools.partial(emit_proj_tile, t,
                                            carry[0], carry[1])
                          for t in range(4 * carry[0], 4 * carry[0] + 4)]
            ats = [emit_attn(0, 0, fill0)]
            for nb in range(k1_start, NKB):
                emit_k_group(1, nb)
            emit_q_group(1, 0)
            # Spread the remaining Q groups across the (otherwise PE-light)
            # steady-state attention calls instead of piling them up front;
            # each Q(j, nb) still strictly precedes its consumer attn(nb, j).
            fill1 = [functools.partial(emit_q_group, 0, 1),
                     functools.partial(emit_q_group, 1, 1)]
            fill1 += [functools.partial(emit_k_group, 2, nb)
                      for nb in range(NKB)]
            ats.append(emit_attn(0, 1, fill1))
            emit_q_group(2, 0)
            fill2 = [functools.partial(emit_q_group, 2, 1),
                     functools.partial(emit_q_group, 0, 2),
                     functools.partial(emit_q_group, 1, 2)]
            ats.append(emit_attn(0, 2, fill2))
            late_q = {(1, 0): [functools.partial(emit_q_group, 2, 2)],
                      (1, 1): [functools.partial(emit_q_group, 0, 3)],
                      (1, 2): [functools.partial(emit_q_group, 1, 3),
                               functools.partial(emit_q_group, 2, 3)]}
            prev = (0, ats)
            for qb in range(1, NQB):
                pj = [functools.partial(emit_proj_tile, t, prev[0], prev[1])
                      for t in range(4 * prev[0], 4 * prev[0] + 4)]
                ats = [emit_attn(qb, 0, pj[0:2] + late_q.get((qb, 0), [])),
                       emit_attn(qb, 1, pj[2:4] + late_q.get((qb, 1), [])),
                       emit_attn(qb, 2, late_q.get((qb, 2), []))]
                prev = (qb, ats)
            carry = prev
        # epilogue of the last rep (earlier reps' tails were pipelined into
        # the following rep's attn(0,0) fillers)
        for d in _pending_div:
            d()
        _pending_div.clear()
        for t in range(4 * carry[0], 4 * carry[0] + 4):
            emit_proj_tile(t, carry[0], carry[1])

    nc.compile()
    return nc


def _prep_inputs(x, attention_mask, qkv_w, q_bias, v_bias, proj_w, proj_b,
                 KT=None):
    f32 = np.float32
    mask = np.asarray(attention_mask) != 0
    counts = mask.sum(axis=1)
    if KT is None:
        KT = max(1, math.ceil(counts.max() / 128))
        KT = 3 * math.ceil(KT / 3)   # K eviction blocks span 3 k-tiles
    NK = KT * 128
    in_maps = []
    for c in range(N_CORES):
        b, h0 = c // 2, (c % 2) * HPC
        rs = slice(h0 * HD, h0 * HD + DPC)
        cnt = int(counts[b])
        xk = np.zeros((NK, DIM), f32)
        xk[:cnt] = x[b][mask[b]]
        xT = np.ascontiguousarray(x[b].T)
        xkT = np.ascontiguousarray(xk.T)
        wqT = np.ascontiguousarray(qkv_w[rs, :].T)
        wkT = np.ascontiguousarray(qkv_w[DIM + h0 * HD: DIM + h0 * HD + DPC, :].T)
        wvT = np.zeros((DIM, VW), f32)
        for h in range(HPC):
            wr = qkv_w[2 * DIM + (h0 + h) * HD: 2 * DIM + (h0 + h) * HD + HD, :]
            wvT[:, h * (HD + 1): h * (HD + 1) + HD] = wr.T

        qb = np.ascontiguousarray(q_bias[rs].reshape(3, 128).T)
        # padding bias: 0 for kept keys, -BIG for padded slots
        pad = np.zeros(NK, f32)
        pad[cnt:] = -BIG
        mb = np.ascontiguousarray(pad.reshape(KT, 128).T)
        pwT = np.ascontiguousarray(proj_w[:, rs].T)
        pb = np.asarray(proj_b, f32) if c % 2 == 0 else np.zeros(DIM, f32)
        pbb = np.ascontiguousarray(np.broadcast_to(pb, (128, DIM)), f32)
        vb_row = np.zeros(VW, f32)
        for h in range(HPC):
            vb_row[h * (HD + 1): h * (HD + 1) + HD] = \
                v_bias[(h0 + h) * HD: (h0 + h + 1) * HD]
            vb_row[h * (HD + 1) + HD] = 1.0
        vbb = np.ascontiguousarray(np.broadcast_to(vb_row, (128, VW)), f32)
        in_maps.append({
            "xT": _tf32_round(xT), "xkT": _tf32_round(xkT),
            "wqT": _tf32_round(wqT), "wkT": _tf32_round(wkT),
            "wvT": _tf32_round(wvT),
            "qbias": qb.astype(f32), "mbias": mb.astype(f32),
            "pwT": _tf32_round(pwT),
            "pbb": pbb, "vbb": vbb,
        })
    return in_maps, KT


def kernel(x, attention_mask, qkv_w, q_bias, v_bias, proj_w, proj_b):
    in_maps, KT = _prep_inputs(x, attention_mask, qkv_w, q_bias, v_bias,
                               proj_w, proj_b)
    if ("nc", KT) not in _CACHE:
        _CACHE[("nc", KT)] = build(KT=KT)
    nc = _CACHE[("nc", KT)]
    _CACHE["nc"] = nc
    res = run_bass_kernel_spmd(nc, in_maps, core_ids=list(range(N_CORES)))
    out = np.empty((B, N, DIM), np.float32)
    for b in range(B):
        out[b] = res.results[2 * b]["out"] + res.results[2 * b + 1]["out"]
    return out


if __name__ == "__main__":
    import reference
    inputs = {k: np.asarray(v) for k, v in reference.setup_inputs().items()}
    got = kernel(**inputs)
    exp = np.asarray(reference.reference(**inputs))
    err = np.abs(got - exp).max()
    rel = err / np.abs(exp).max()
    print("max abs err:", err, "rel:", rel)


# revision 29
# speedup vs baseline: 2.2294x; 2.2294x over previous
"""Trainium2 Bass kernel for multi-head attention (B=4, N=2048, DIM=768, H=12).

Sharding: 8 cores; core c handles batch b = c//2 and heads h0 = 6*(c%2) .. h0+5
(tensor-parallel over heads within a batch pair). Each core computes a partial
projection output; the host sums the two partials per batch (proj_b is folded
into the even core's partial on device).

Key compaction: attention_mask zeroes ~half the keys, and masked keys
contribute exactly 0 after softmax. The host gathers only the unmasked keys'
x rows (padded to KT*128), so K/V/S/exp/PV all shrink ~2x. Padded key slots
get an exp bias of -BIG via mbias (per-partition ACT bias), so they vanish
from both the numerator and the ones-column denominator.

Device-side layout (per core):
  - Q^T: [384, 2048] f32r as 3 SBUF tiles of [128, 2048] (2 heads per tile);
    K^T: [384, NK] likewise. V: [NK, 390] f32r as KT tiles of [128, 390];
    per head 65 cols = [V | 1], so the attention matmul also produces the
    softmax denominator in row 64.
  - S^T = K @ Q^T per (k-tile, head, q-block) in PSUM (f32r, 64-contraction);
    exp with per-partition padding bias + 1/8 scale fused in one ACT
    instruction over [128, 1024] (2 heads x 512-q-block, same k-tile).
  - P^T @ V f32r with accumulating matmuls; accumulator copied to SBUF to
    free its PSUM bank, then divided by the denominator (reciprocal +
    GPSIMD partition-broadcast + multiply).
  - proj: out[n, e] = attnT.T @ proj_w_slice.T in f32r; proj_b and v_bias
    added during DVE evictions; partials summed host-side across core pairs.
  - PSUM (8 banks): S tiles 2x2 banks (double-buffered, S emitted one
    iteration ahead to keep ACT fed), 2 attention accumulators, 2 banks for
    the QKV/V/proj rotation. All QKV/V/proj matmul groups stream just-in-time
    into the attention iterations as low-priority fillers.
  - Schedule: the kernel is paced by the ACT engine (exp is ACT-only; ~110us
    of irreducible exp work/core). The serial prologue is only {K0 block 0,
    Q00, V0-V2} (K0 b1/b2 ride as fillers ahead of their S consumers); the
    remaining Q groups spread into the PE-light steady-state calls; and the
    per-rep tail (last division + final proj tiles) is software-pipelined
    across the rep boundary (division crosses via _pending_div, proj tiles
    ride as post-loop fillers of the next rep's first attention call) so
    ACT stays busy across boundaries.
proj matmuls run as float32r (TF32); host pre-rounds f32r inputs to TF32.

(Tried and rejected: fp8e4 DoubleRow for S — the ISA-mandated [32, 2, m]
weight repack needs cross-partition DMAs and DR disables fast weight load,
so real hardware ran 44us SLOWER than f32r despite 2x fewer matmul cycles.
fp8 P/V for the attention-value matmul fails the 2e-2 error budget.)
"""

import math
import numpy as np
from contextlib import ExitStack

import concourse.bass as bass
import concourse.tile as tile
from concourse import bacc, mybir
from concourse.alu_op_type import AluOpType
from concourse.bass_utils import run_bass_kernel_spmd

N_CORES = 8
B, N, DIM = 4, 2048, 768
NHEADS, HD = 12, 64
HPC = 6              # heads per core
DPC = HPC * HD       # 384 channels per core
CC = DIM // 128      # 6 contraction chunks of 128
QB = 512             # q block width
NQB = N // QB        # 4 q blocks
KB = 384             # k eviction block width
VW = HPC * (HD + 1)  # V width incl per-head ones column (390)
BIG = 80.0
F32 = mybir.dt.float32
F32R = mybir.dt.float32r
BF16 = mybir.dt.bfloat16
FP8 = mybir.dt.float8e4
AF = mybir.ActivationFunctionType
DR = mybir.MatmulPerfMode.DoubleRow

_CACHE = {}


def _tf32_round(a: np.ndarray) -> np.ndarray:
    """Round fp32 to TF32 (10-bit mantissa, round-to-nearest-even)."""
    u = np.ascontiguousarray(a, np.float32).view(np.uint32)
    r = (u + np.uint32(0x0FFF) + ((u >> np.uint32(13)) & np.uint32(1))) \
        & np.uint32(0xFFFFE000)
    return r.view(np.float32)


def _bf16(a: np.ndarray) -> np.ndarray:
    import ml_dtypes
    return np.ascontiguousarray(np.asarray(a, np.float32)).astype(
        ml_dtypes.bfloat16)


def build(reps=1, KT=9):
    NK = KT * 128        # padded kept-key count
    NKB = NK // KB       # k eviction blocks
    nc = bacc.Bacc("TRN2", target_bir_lowering=False, debug=False,
                   num_devices=N_CORES)

    xT = nc.dram_tensor("xT", [DIM, N], F32R, kind="ExternalInput").ap()
    xkT = nc.dram_tensor("xkT", [DIM, NK], F32R, kind="ExternalInput").ap()
    wqT = nc.dram_tensor("wqT", [DIM, DPC], F32R, kind="ExternalInput").ap()
    wkT = nc.dram_tensor("wkT", [DIM, DPC], F32R, kind="ExternalInput").ap()
    wvT = nc.dram_tensor("wvT", [DIM, VW], F32R, kind="ExternalInput").ap()
    qbias = nc.dram_tensor("qbias", [128, 3], F32, kind="ExternalInput").ap()
    mbias = nc.dram_tensor("mbias", [128, KT], F32, kind="ExternalInput").ap()
    pwT = nc.dram_tensor("pwT", [DPC, DIM], F32R, kind="ExternalInput").ap()
    pbb = nc.dram_tensor("pbb", [128, DIM], F32, kind="ExternalInput").ap()
    vbb = nc.dram_tensor("vbb", [128, VW], F32, kind="ExternalInput").ap()
    out = nc.dram_tensor("out", [N, DIM], F32, kind="ExternalOutput").ap()

    with tile.TileContext(nc) as tc, ExitStack() as ctx:
        # ---- pools ----
        # One shared PSUM pool, exactly 8 banks:
        #   tag "s":  [128, 1024] x2 bufs = 4 banks (S^T tiles)
        #   tag "o":  [65, 512]   x2 bufs = 2 banks (attention accumulators)
        #   tag "pj": [128, <=512] x2     = 2 banks (QK/V/proj psum rotation)
        psum = ctx.enter_context(tc.tile_pool(name="psum", bufs=2, space="PSUM"))
        persist = ctx.enter_context(tc.tile_pool(name="persist", bufs=1))
        qt_sb = [persist.tile([128, N], F32R, tag=f"qt{j}", name=f"qt{j}")
                 for j in range(3)]
        kt_sb = [persist.tile([128, NK], F32R, tag=f"kt{j}", name=f"kt{j}")
                 for j in range(3)]
        v_sb = [persist.tile([128, VW], F32R, tag=f"v{t}", name=f"v{t}")
                for t in range(KT)]
        consts = ctx.enter_context(tc.tile_pool(name="consts", bufs=1))
        qb_sb = consts.tile([128, 3], F32, tag="qb", name="qb")
        mb_sb = consts.tile([128, KT], F32, tag="mb", name="mb")
        pbb_sb = consts.tile([128, DIM], F32, tag="pbb", name="pbb")
        vbb_sb = consts.tile([128, VW], F32, tag="vbb", name="vbb")
        pw_sb = [consts.tile([128, DIM], F32R, tag=f"pw{j}", name=f"pw{j}")
                 for j in range(3)]
        xw = ctx.enter_context(tc.tile_pool(name="xw", bufs=1))
        x_sb = [xw.tile([128, N], F32R, tag=f"x{c}", name=f"x{c}")
                for c in range(CC)]
        xk_sb = [xw.tile([128, NK], F32R, tag=f"xk{c}", name=f"xk{c}")
                 for c in range(CC)]
        wq_sb = [xw.tile([128, DPC], F32R, tag=f"wq{c}", name=f"wq{c}")
                 for c in range(CC)]
        wk_sb = [xw.tile([128, DPC], F32R, tag=f"wk{c}", name=f"wk{c}")
                 for c in range(CC)]
        wv_sb = [xw.tile([128, VW], F32R, tag=f"wv{c}", name=f"wv{c}")
                 for c in range(CC)]
        ppool = ctx.enter_context(tc.tile_pool(name="ppool", bufs=4))
        atpool = ctx.enter_context(tc.tile_pool(name="atpool", bufs=1))
        dpool = ctx.enter_context(tc.tile_pool(name="dpool", bufs=2))
        opool = ctx.enter_context(tc.tile_pool(name="opool", bufs=2))

        # ---- input DMAs ----
        nc.sync.dma_start(qb_sb[:], qbias)
        nc.sync.dma_start(mb_sb[:], mbias)
        nc.sync.dma_start(pbb_sb[:], pbb)
        nc.sync.dma_start(vbb_sb[:], vbb)
        for j in range(3):
            nc.sync.dma_start(pw_sb[j][:], pwT[j * 128:(j + 1) * 128, :])
        for c in range(CC):
            nc.sync.dma_start(xk_sb[c][:], xkT[c * 128:(c + 1) * 128, :])
            nc.sync.dma_start(x_sb[c][:], xT[c * 128:(c + 1) * 128, :])
        for c in range(CC):
            nc.sync.dma_start(wk_sb[c][:], wkT[c * 128:(c + 1) * 128, :])
            nc.sync.dma_start(wq_sb[c][:], wqT[c * 128:(c + 1) * 128, :])
        for c in range(CC):
            nc.sync.dma_start(wv_sb[c][:], wvT[c * 128:(c + 1) * 128, :])

        _qk_open = {}

        def emit_q_half(j, nb, half):
            # half of a [128, 512] Q chunk (3 contraction steps) through
            # the 1-bank pj rotation; finer PE granules between attention iters
            dsl = bass.ts(j, 128)
            nsl = bass.ts(nb, QB)
            if half == 0:
                ps = psum.tile([128, QB], F32, tag="pj", name=f"q{j}_{nb}")
                _qk_open[("q", j, nb)] = ps
            else:
                ps = _qk_open.pop(("q", j, nb))
            for c in range(3 * half, 3 * half + 3):
                nc.tensor.matmul(ps[:], wq_sb[c][:, dsl], x_sb[c][:, nsl],
                                 start=(c == 0), stop=(c == CC - 1))
            if half == 1:
                nc.vector.tensor_scalar(qt_sb[j][:, nsl], ps[:],
                                        qb_sb[:, j:j + 1], None,
                                        op0=AluOpType.add)

        def emit_q_group(j, nb):
            emit_q_half(j, nb, 0)
            emit_q_half(j, nb, 1)

        def emit_k_half(j, nb, half):
            dsl = bass.ts(j, 128)
            nsl = bass.ts(nb, KB)
            if half == 0:
                ps = psum.tile([128, KB], F32, tag="pj", name=f"k{j}_{nb}")
                _qk_open[("k", j, nb)] = ps
            else:
                ps = _qk_open.pop(("k", j, nb))
            for c in range(3 * half, 3 * half + 3):
                nc.tensor.matmul(ps[:], wk_sb[c][:, dsl], xk_sb[c][:, nsl],
                                 start=(c == 0), stop=(c == CC - 1))
            if half == 1:
                nc.vector.tensor_copy(kt_sb[j][:, nsl], ps[:])

        def emit_k_group(j, nb):
            emit_k_half(j, nb, 0)
            emit_k_half(j, nb, 1)

        def emit_v(t):
            tsl = bass.ts(t, 128)
            psv = psum.tile([128, VW], F32, tag="pj", name=f"v{t}")
            for c in range(CC):
                nc.tensor.matmul(psv[:], xk_sb[c][:, tsl], wv_sb[c][:],
                                 start=(c == 0), stop=(c == CC - 1))
            nc.vector.tensor_tensor(v_sb[t][:], psv[:], vbb_sb[:],
                                    op=AluOpType.add)

        _pending_div = []

        def emit_attn(qb, j, fillers=()):
            # heads 2j, 2j+1 over q block qb: S^T -> exp -> P^T V -> divide
            fillers = list(fillers)
            qsl = bass.ds(qb * QB, QB)
            at_t = atpool.tile([128, QB], F32R, tag=f"at{j}", name=f"at{qb}_{j}")
            o_ps = [psum.tile([HD + 1, QB], F32, tag="o", name=f"o{qb}_{j}_{i}")
                    for i in range(2)]
            s_tiles = {}

            def emit_s(kt):
                ksl = bass.ts(kt, 128)
                s = psum.tile([128, 2 * QB], F32, tag="s",
                              name=f"s{qb}_{j}_{kt}")
                for hh in range(2):
                    hsl = bass.ds(hh * 64, 64)
                    nc.tensor.matmul(s[:, bass.ts(hh, QB)],
                                     kt_sb[j][hsl, ksl], qt_sb[j][hsl, qsl],
                                     start=True, stop=True)
                s_tiles[kt] = s

            emit_s(0)
            for d in _pending_div:
                d()
            _pending_div.clear()
            for kt in range(KT):
                if kt + 1 < KT:
                    emit_s(kt + 1)   # one iteration ahead: keeps ACT fed
                s = s_tiles.pop(kt)
                p = ppool.tile([128, 2 * QB], F32R, tag="p", name=f"p{qb}_{j}_{kt}")
                nc.scalar.activation(p[:], s[:], AF.Exp,
                                     bias=mb_sb[:, kt:kt + 1], scale=0.125)
                for hh in range(2):
                    vsl = bass.ds((2 * j + hh) * (HD + 1), HD + 1)
                    nc.tensor.matmul(o_ps[hh], v_sb[kt][:, vsl],
                                     p[:, bass.ts(hh, QB)],
                                     start=(kt == 0), stop=(kt == KT - 1))
                if fillers:
                    fillers.pop(0)()
            for f in fillers:
                f()
            def _division():
                for hh in range(2):
                    ocp = dpool.tile([HD + 1, QB], F32, tag="ocp", name="ocp")
                    nc.vector.tensor_copy(ocp[:], o_ps[hh])  # frees the bank
                    rd = dpool.tile([1, QB], F32, tag="rd", name="rd")
                    bc = dpool.tile([64, QB], F32, tag="bc", name="bc")
                    nc.vector.reciprocal(rd[:], ocp[HD:HD + 1, :])
                    nc.gpsimd.partition_broadcast(bc[:], rd[:])
                    nc.vector.tensor_tensor(at_t[bass.ds(hh * 64, 64), :],
                                            ocp[0:HD, :], bc[:],
                                            op=AluOpType.mult)
            _pending_div.append(_division)
            return at_t

        def emit_proj_tile(t, qb, at_tiles):
            tsl = bass.ts(t, 128)
            po = [psum.tile([128, 384], F32, tag="pj", name=f"po{t}_{e}")
                  for e in range(2)]
            for e in range(2):
                esl = bass.ts(e, 384)
                for j in range(3):
                    nc.tensor.matmul(po[e],
                                     at_tiles[j][:, bass.ts(t - 4 * qb, 128)],
                                     pw_sb[j][:, esl],
                                     start=(j == 0), stop=(j == 2))
            for e in range(2):
                ot = opool.tile([128, 384], F32, tag="ot", name=f"ot{t}_{e}")
                nc.vector.tensor_tensor(ot[:], po[e],
                                        pbb_sb[:, bass.ts(e, 384)],
                                        op=AluOpType.add)
                nc.sync.dma_start(out[tsl, bass.ts(e, 384)], ot[:])

        # ---- emission: sprinkle QKV/proj groups into the attention stream
        import functools
        carry = None
        for _rep in range(reps):
            if KT == 9:
                # minimal serial prologue: K0 block 0 only, Q0 block 0,
                # V0-V2; K0 blocks 1/2 stream as fillers ahead of their
                # first consumers (S(kt=3) emitted at iteration 2, S(kt=6)
                # at iteration 5; V(t) consumed at iteration t).
                emit_k_group(0, 0)
                emit_q_group(0, 0)
                for t in range(3):
                    emit_v(t)
                fill0 = [functools.partial(emit_k_group, 0, 1),
                         functools.partial(emit_v, 3),
                         functools.partial(emit_v, 4),
                         functools.partial(emit_k_group, 0, 2),
                         functools.partial(emit_v, 5),
                         functools.partial(emit_v, 6),
                         functools.partial(emit_v, 7),
                         functools.partial(emit_v, 8),
                         functools.partial(emit_k_group, 1, 0)]
                k1_start = 1
            else:
                # conservative prologue for other KT
                for nb in range(NKB):
                    emit_k_group(0, nb)
                emit_q_group(0, 0)
                emit_v(0)
                emit_v(1)
                fill0 = [functools.partial(emit_v, t) for t in range(2, KT)]
                fill0 += [functools.partial(emit_k_group, 1, nb)
                          for nb in range(2)]
                k1_start = 2
            if carry is not None:
                # software-pipelined rep boundary: the previous rep's final
                # proj tiles run as post-loop fillers here (PE-slack window
                # while ACT works through this attn's exps); its last
                # division crosses via _pending_div into this attn's start.
                fill0 += [functools.partial(emit_proj_tile, t,
                                            carry[0], carry[1])
                          for t in range(4 * carry[0], 4 * carry[0] + 4)]
            ats = [emit_attn(0, 0, fill0)]
            for nb in range(k1_start, NKB):
                emit_k_group(1, nb)
            emit_q_group(1, 0)
            # Spread the remaining Q groups across the (otherwise PE-light)
            # steady-state attention calls instead of piling them up front;
            # each Q(j, nb) still strictly precedes its consumer attn(nb, j).
            fill1 = [functools.partial(emit_q_group, 0, 1),
                     functools.partial(emit_q_group, 1, 1)]
            fill1 += [functools.partial(emit_k_group, 2, nb)
                      for nb in range(NKB)]
            ats.append(emit_attn(0, 1, fill1))
            emit_q_group(2, 0)
            fill2 = [functools.partial(emit_q_group, 2, 1),
                     functools.partial(emit_q_group, 0, 2),
                     functools.partial(emit_q_group, 1, 2)]
            ats.append(emit_attn(0, 2, fill2))
            late_q = {(1, 0): [functools.partial(emit_q_group, 2, 2)],
                      (1, 1): [functools.partial(emit_q_group, 0, 3)],
                      (1, 2): [functools.partial(emit_q_group, 1, 3),
                               functools.partial(emit_q_group, 2, 3)]}
            prev = (0, ats)
            for qb in range(1, NQB):
                pj = [functools.partial(emit_proj_tile, t, prev[0], prev[1])
                      for t in range(4 * prev[0], 4 * prev[0] + 4)]
                ats = [emit_attn(qb, 0, pj[0:2] + late_q.get((qb, 0), [])),
                       emit_attn(qb, 1, pj[2:4] + late_q.get((qb, 1), [])),
                       emit_attn(qb, 2, late_q.get((qb, 2), []))]
                prev = (qb, ats)
            carry = prev
        # epilogue of the last rep (earlier reps' tails were pipelined into
        # the following rep's attn(0,0) fillers)
        for d in _pending_div:
            d()
        _pending_div.clear()
        for t in range(4 * carry[0], 4 * carry[0] + 4):
            emit_proj_tile(t, carry[0], carry[1])

    nc.compile()
    return nc


def _prep_inputs(x, attention_mask, qkv_w, q_bias, v_bias, proj_w, proj_b,
                 KT=None):
    f32 = np.float32
    mask = np.asarray(attention_mask) != 0
    counts = mask.sum(axis=1)
    if KT is None:
        KT = max(1, math.ceil(counts.max() / 128))
        KT = 3 * math.ceil(KT / 3)   # K eviction blocks span 3 k-tiles
    NK = KT * 128
    in_maps = []
    for c in range(N_CORES):
        b, h0 = c // 2, (c % 2) * HPC
        rs = slice(h0 * HD, h0 * HD + DPC)
        cnt = int(counts[b])
        xk = np.zeros((NK, DIM), f32)
        xk[:cnt] = x[b][mask[b]]
        xT = np.ascontiguousarray(x[b].T)
        xkT = np.ascontiguousarray(xk.T)
        wqT = np.ascontiguousarray(qkv_w[rs, :].T)
        wkT = np.ascontiguousarray(qkv_w[DIM + h0 * HD: DIM + h0 * HD + DPC, :].T)
        wvT = np.zeros((DIM, VW), f32)
        for h in range(HPC):
            wr = qkv_w[2 * DIM + (h0 + h) * HD: 2 * DIM + (h0 + h) * HD + HD, :]
            wvT[:, h * (HD + 1): h * (HD + 1) + HD] = wr.T

        qb = np.ascontiguousarray(q_bias[rs].reshape(3, 128).T)
        # padding bias: 0 for kept keys, -BIG for padded slots
        pad = np.zeros(NK, f32)
        pad[cnt:] = -BIG
        mb = np.ascontiguousarray(pad.reshape(KT, 128).T)
        pwT = np.ascontiguousarray(proj_w[:, rs].T)
        pb = np.asarray(proj_b, f32) if c % 2 == 0 else np.zeros(DIM, f32)
        pbb = np.ascontiguousarray(np.broadcast_to(pb, (128, DIM)), f32)
        vb_row = np.zeros(VW, f32)
        for h in range(HPC):
            vb_row[h * (HD + 1): h * (HD + 1) + HD] = \
                v_bias[(h0 + h) * HD: (h0 + h + 1) * HD]
            vb_row[h * (HD + 1) + HD] = 1.0
        vbb = np.ascontiguousarray(np.broadcast_to(vb_row, (128, VW)), f32)
        in_maps.append({
            "xT": _tf32_round(xT), "xkT": _tf32_round(xkT),
            "wqT": _tf32_round(wqT), "wkT": _tf32_round(wkT),
            "wvT": _tf32_round(wvT),
            "qbias": qb.astype(f32), "mbias": mb.astype(f32),
            "pwT": _tf32_round(pwT),
            "pbb": pbb, "vbb": vbb,
        })
    return in_maps, KT


def kernel(x, attention_mask, qkv_w, q_bias, v_bias, proj_w, proj_b):
    in_maps, KT = _prep_inputs(x, attention_mask, qkv_w, q_bias, v_bias,
                               proj_w, proj_b)
    if ("nc", KT) not in _CACHE:
        _CACHE[("nc", KT)] = build(KT=KT)
    nc = _CACHE[("nc", KT)]
    _CACHE["nc"] = nc
    res = run_bass_kernel_spmd(nc, in_maps, core_ids=list(range(N_CORES)))
    out = np.empty((B, N, DIM), np.float32)
    for b in range(B):
        out[b] = res.results[2 * b]["out"] + res.results[2 * b + 1]["out"]
    return out


if __name__ == "__main__":
    import reference
    inputs = {k: np.asarray(v) for k, v in reference.setup_inputs().items()}
    got = kernel(**inputs)
    exp = np.asarray(reference.reference(**inputs))
    err = np.abs(got - exp).max()
    rel = err / np.abs(exp).max()
    print("max abs err:", err, "rel:", rel)
